# revision 9
# baseline (speedup 1.0000x reference)
"""Trainium2 Bass kernel for Llama4TextExperts — v2b (TT=1024 redesign).

Per-core (1 expert): T=2048 tokens, H=2048, D=4096, all operands bf16.

  - 2 t-tiles of TT=1024 tokens; weights stream from HBM once per t-tile
    (50MB/t-tile, all bf16; half the traffic of TT=512).
  - stage 1 per d-tile (128 wide): 4 PSUM chains (g/u x t-half), pairwise
    interleaved per h so consecutive matmuls share the stationary weight.
    silu on ScalarE, f = silu(g)*u on VectorE -> f[dt] [128(d) x 1024(t)]
    bf16 SBUF tiles.
  - stage 2: wd stationary ([128(d),128(h)] bf16 -> FWL), f moving:
    psum_yT[h-tile, t-half] [128(h) x 512(t)] accumulates 32 d-tiles with
    the two t-half chains interleaved. Output is yT [H, T]; host
    transposes back (free - HW time is what counts).
  - PSUM: stage-1 4 banks (bufs=1) + stage-2 2x2 banks = 8.
  - xT loads split into 4 chunks per t-tile for a fast start.
"""

import os
import sys

for _p in ("/opt/trn_rl_repo",):
    if _p not in sys.path and os.path.isdir(_p):
        sys.path.insert(0, _p)

import numpy as np
from ml_dtypes import bfloat16 as bf16

E = 8
T = 2048
H = 2048
D = 4096

_CACHE = {}


def _build_bass(H_=H, D_=D, T_=T, TT=1024, split_waits=True, act="Silu"):
    import concourse.bass as bass
    import concourse.mybir as mybir
    from concourse.tile import TileContext

    f32 = mybir.dt.float32
    bf16 = mybir.dt.bfloat16
    P = 128
    N_H = H_ // P            # h-chunks for stage-1 contraction (16)
    N_D = D_ // P            # d-tiles (32)
    N_TT = T_ // TT          # t-tiles (2)
    HTT = TT // 2            # t-half width (512) = moving free dim
    WGD = 256                # wg/wu d-width per load (2 d-tiles)
    XCH = 4                  # h-chunks of x per load chunk
    N_XC = N_H // XCH        # 4 x-load chunks per t-tile

    nc = bass.Bass(trn_type="TRN2")

    xT = nc.declare_dram_parameter("xT", [H_, T_], bf16, isOutput=False)
    wg = nc.declare_dram_parameter("wg", [H_, D_], bf16, isOutput=False)
    wu = nc.declare_dram_parameter("wu", [H_, D_], bf16, isOutput=False)
    wd = nc.declare_dram_parameter("wd", [D_, H_], bf16, isOutput=False)
    yT = nc.declare_dram_parameter("yT", [H_, T_], f32, isOutput=True)

    xT_r = xT[:].rearrange("(n p) t -> p n t", p=P)    # [128, N_H, T]
    wg_r = wg[:].rearrange("(n p) d -> p n d", p=P)    # [128, N_H, D]
    wu_r = wu[:].rearrange("(n p) d -> p n d", p=P)
    wd_r = wd[:].rearrange("(n p) h -> p n h", p=P)    # [128, N_D, H]
    yT_r = yT[:].rearrange("(n p) t -> p n t", p=P)    # [128, N_H, T]

    actf = getattr(mybir.ActivationFunctionType, act)

    with TileContext(nc) as tc:
        with (
            tc.tile_pool(name="xpool", bufs=N_XC + 1) as xpool,
            tc.tile_pool(name="wpool", bufs=2) as wpool,
            tc.tile_pool(name="wdpool", bufs=3) as wdpool,
            tc.tile_pool(name="fpool", bufs=N_D + 2) as fpool,
            tc.tile_pool(name="spool", bufs=4) as spool,
            tc.tile_pool(name="ypool", bufs=4) as ypool,
            tc.tile_pool(name="pgu", bufs=1, space="PSUM") as pgu,
            tc.tile_pool(name="py", bufs=2, space="PSUM") as py,
        ):
            for tt in range(N_TT):
                tsl = slice(tt * TT, (tt + 1) * TT)
                # ---- load xT[:, t-tile] in N_XC chunks of [128, XCH, TT]
                x_ts = []
                for xc in range(N_XC):
                    x_t = xpool.tile([P, XCH, TT], bf16, tag="x")
                    nc.sync.dma_start(
                        out=x_t, in_=xT_r[:, xc * XCH:(xc + 1) * XCH, tsl])
                    x_ts.append(x_t)

                # ---- stage 1: gate/up + swiglu, d-tile at a time
                f_tiles = []
                for dt in range(N_D):
                    dw = dt % (WGD // P)
                    if dw == 0:
                        dsl = slice(dt * P, dt * P + WGD)
                        wg_t = wpool.tile([P, N_H, WGD], bf16, tag="wg")
                        wu_t = wpool.tile([P, N_H, WGD], bf16, tag="wu")
                        nc.sync.dma_start(out=wg_t, in_=wg_r[:, :, dsl])
                        nc.sync.dma_start(out=wu_t, in_=wu_r[:, :, dsl])
                    psum_g0 = pgu.tile([P, HTT], f32, tag="pg0")
                    psum_g1 = pgu.tile([P, HTT], f32, tag="pg1")
                    psum_u0 = pgu.tile([P, HTT], f32, tag="pu0")
                    psum_u1 = pgu.tile([P, HTT], f32, tag="pu1")
                    for h in range(N_H):
                        w_ap = wg_t[:, h, dw * P:(dw + 1) * P]
                        x_ap = x_ts[h // XCH]
                        nc.tensor.matmul(
                            psum_g0, lhsT=w_ap,
                            rhs=x_ap[:, h % XCH, 0:HTT],
                            start=(h == 0), stop=(h == N_H - 1),
                        )
                        nc.tensor.matmul(
                            psum_g1, lhsT=w_ap,
                            rhs=x_ap[:, h % XCH, HTT:TT],
                            start=(h == 0), stop=(h == N_H - 1),
                        )
                    for h in range(N_H):
                        w_ap = wu_t[:, h, dw * P:(dw + 1) * P]
                        x_ap = x_ts[h // XCH]
                        nc.tensor.matmul(
                            psum_u0, lhsT=w_ap,
                            rhs=x_ap[:, h % XCH, 0:HTT],
                            start=(h == 0), stop=(h == N_H - 1),
                        )
                        nc.tensor.matmul(
                            psum_u1, lhsT=w_ap,
                            rhs=x_ap[:, h % XCH, HTT:TT],
                            start=(h == 0), stop=(h == N_H - 1),
                        )
                    f_t = fpool.tile([P, TT], bf16, tag="f")
                    s0 = spool.tile([P, HTT], f32, tag="s")
                    nc.scalar.activation(out=s0, in_=psum_g0, func=actf)
                    nc.vector.tensor_mul(f_t[:, 0:HTT], s0, psum_u0)
                    s1 = spool.tile([P, HTT], f32, tag="s")
                    nc.scalar.activation(out=s1, in_=psum_g1, func=actf)
                    nc.vector.tensor_mul(f_t[:, HTT:TT], s1, psum_u1)
                    f_tiles.append(f_t)

                # ---- stage 2: yT[h, t] = sum_d wd[d, h] * f[d, t]
                # wd stationary (FWL), f moving; two t-half chains
                # interleaved so consecutive matmuls share the weight.
                for ht in range(N_H):
                    wd_t = wdpool.tile([P, N_D, P], bf16, tag="wd")
                    nc.sync.dma_start(
                        out=wd_t,
                        in_=wd_r[:, :, ht * P:(ht + 1) * P],
                    )
                    psum_y0 = py.tile([P, HTT], f32, tag="py0")
                    psum_y1 = py.tile([P, HTT], f32, tag="py1")
                    for dt in range(N_D):
                        w_ap = wd_t[:, dt, :]
                        nc.tensor.matmul(
                            psum_y0, lhsT=w_ap,
                            rhs=f_tiles[dt][:, 0:HTT],
                            start=(dt == 0), stop=(dt == N_D - 1),
                        )
                        nc.tensor.matmul(
                            psum_y1, lhsT=w_ap,
                            rhs=f_tiles[dt][:, HTT:TT],
                            start=(dt == 0), stop=(dt == N_D - 1),
                        )
                    y_sb = ypool.tile([P, TT], f32, tag="y")
                    nc.vector.tensor_copy(y_sb[:, 0:HTT], psum_y0)
                    nc.vector.tensor_copy(y_sb[:, HTT:TT], psum_y1)
                    nc.sync.dma_start(
                        out=yT_r[:, ht, tsl],
                        in_=y_sb,
                    )
    if split_waits:
        _split_matmul_waits(nc)
    return nc


def _split_matmul_waits(nc):
    """walrus splits Matmult into LDW+MM and moves the Matmult's sync
    waits onto the generated LW struct, which has room for only one wait.
    Hoist every Matmult's waits onto a PE InstNoOp inserted just before it."""
    import concourse.mybir as mybir

    for f in nc.m.functions:
        for bb in f.blocks:
            insts = list(bb.instructions)
            out = []
            n_nops = 0
            for ins in insts:
                si = ins.sync_info
                tname = type(ins).__name__
                if (
                    si is not None
                    and len(si.on_wait) > (1 if tname != "InstMatmult" else 0)
                ):
                    keep = [] if tname == "InstMatmult" else [si.on_wait[-1]]
                    hoist = si.on_wait if tname == "InstMatmult" else si.on_wait[:-1]
                    for i, w in enumerate(hoist):
                        nop = mybir.InstNoOp(
                            name=f"{ins.name}-waitnop{i}",
                            engine=ins.engine,
                            ins=[],
                            outs=[],
                            sync_info=mybir.SyncInfo(
                                on_wait=[w], on_update=[]
                            ),
                        )
                        out.append(nop)
                        n_nops += 1
                    ins.sync_info = mybir.SyncInfo(
                        on_wait=keep, on_update=list(si.on_update)
                    )
                out.append(ins)
            if n_nops:
                bb.instructions = out


def make_in_maps(hidden_states, gate_proj, up_proj, down_proj):
    hs = np.ascontiguousarray(hidden_states, dtype=np.float32).reshape(E, T, H)
    in_maps = []
    for e in range(E):
        in_maps.append({
            "xT": np.ascontiguousarray(hs[e].T).astype(bf16),
            "wg": np.ascontiguousarray(gate_proj[e], dtype=np.float32).astype(bf16),
            "wu": np.ascontiguousarray(up_proj[e], dtype=np.float32).astype(bf16),
            "wd": np.ascontiguousarray(down_proj[e], dtype=np.float32).astype(bf16),
        })
    return in_maps


def kernel(hidden_states, gate_proj, up_proj, down_proj):
    from concourse.bass_utils import run_bass_kernel_spmd

    in_maps = make_in_maps(hidden_states, gate_proj, up_proj, down_proj)
    if "nc" not in _CACHE:
        _CACHE["nc"] = _build_bass()
    nc = _CACHE["nc"]

    res = run_bass_kernel_spmd(nc, in_maps, core_ids=list(range(E)))
    out = np.concatenate(
        [np.ascontiguousarray(res.results[e]["yT"].T) for e in range(E)], axis=0)
    return out.astype(np.float32)


if __name__ == "__main__":
    nc = _build_bass()
    print("built ok, instructions:", len(nc.inst_map))


# revision 12
# speedup vs baseline: 1.0029x; 1.0029x over previous
"""Trainium2 Bass kernel for Llama4TextExperts (MoE expert MLP chain).

Problem: E=8 experts, T=2048 tokens/expert, H=2048 hidden, D=4096 intermediate.
  hs (E*T, H) -> per expert e: g = hs_e @ Wg_e; u = hs_e @ Wu_e;
  f = u * silu(g); y_e = f @ Wd_e  -> out (E*T, H), all fp32.

Sharding: expert-parallel, 1 expert per NeuronCore (8 cores).

Per-core kernel design (all operands bf16, PSUM accumulate fp32):
  - Host pre-transposes hs_e -> xT [H, T] so the stage-1 moving operand has
    the contraction dim (H) on partitions.
  - Loop over T in tiles of TT=512 tokens:
      stage 1: for each of 32 d-tiles (128 wide): psum_g/psum_u accumulate
        16 matmuls over h-chunks (lhsT = W[h,d] 128x128 stationary bf16,
        rhs = xT[h, t] 128x512 moving bf16). silu on ScalarE,
        f = silu(g)*u on VectorE -> f_T[d] SBUF tiles [128(d) x 512(t)] bf16.
      stage 2: for each of 4 h-chunks (512 wide): for each of 4 t-subtiles
        (128): psum_y accumulates 32 matmuls over d (lhsT = f_T[d][:, ts]
        128x128 bf16 -> FWL fast weight load, rhs = wd[d, h] 128x512 bf16
        moving) -> copy -> DMA out.
  - Weights stream from HBM once per t-tile (50MB/t-tile, all bf16); DMA
    overlaps PE via double-buffered pools.
  - xT loads split into 4 chunks per t-tile so the first matmul can start
    after ~1.5MB of DMA instead of ~4MB.
"""

import os
import sys

for _p in ("/opt/trn_rl_repo",):
    if _p not in sys.path and os.path.isdir(_p):
        sys.path.insert(0, _p)

import numpy as np
from ml_dtypes import bfloat16 as bf16

E = 8
T = 2048
H = 2048
D = 4096

_CACHE = {}


def _build_bass(H_=H, D_=D, T_=T, TT=512, split_waits=True, act="Silu"):
    """Build the single-core Bass module (same program for all 8 cores)."""
    import concourse.bass as bass
    import concourse.mybir as mybir
    from concourse.tile import TileContext

    f32 = mybir.dt.float32
    bf16 = mybir.dt.bfloat16
    P = 128
    N_H = H_ // P            # h-chunks (16)
    N_D = D_ // P            # d-tiles (32)
    N_TT = T_ // TT          # t-tiles (4)
    TS = TT // P             # t-subtiles per t-tile (4)
    HC = 512                 # stage-2 output h-chunk width
    N_HC = H_ // HC          # 4
    WGD = 256                # wg/wu d-width per load (2 d-tiles)
    WD_DCH = 8               # wd d-chunks per load tile
    XCH = 4                  # h-chunks of x per load chunk
    N_XC = N_H // XCH        # 4 x-load chunks per t-tile

    nc = bass.Bass(trn_type="TRN2")

    xT = nc.declare_dram_parameter("xT", [H_, T_], bf16, isOutput=False)
    wg = nc.declare_dram_parameter("wg", [H_, D_], bf16, isOutput=False)
    wu = nc.declare_dram_parameter("wu", [H_, D_], bf16, isOutput=False)
    wd = nc.declare_dram_parameter("wd", [D_, H_], bf16, isOutput=False)
    y = nc.declare_dram_parameter("y", [T_, H_], f32, isOutput=True)

    xT_r = xT[:].rearrange("(n p) t -> p n t", p=P)    # [128, N_H, T]
    wg_r = wg[:].rearrange("(n p) d -> p n d", p=P)    # [128, N_H, D]
    wu_r = wu[:].rearrange("(n p) d -> p n d", p=P)
    wd_r = wd[:].rearrange("(n p) h -> p n h", p=P)    # [128, N_D, H]
    y_r = y[:].rearrange("(n p) h -> p n h", p=P)      # [128, T//128, H]

    with TileContext(nc) as tc:
        with (
            tc.tile_pool(name="xpool", bufs=N_XC + 1) as xpool,
            tc.tile_pool(name="wpool", bufs=2) as wpool,
            tc.tile_pool(name="wdpool", bufs=4) as wdpool,
            tc.tile_pool(name="fpool", bufs=N_D + 2) as fpool,
            tc.tile_pool(name="spool", bufs=3) as spool,
            tc.tile_pool(name="ypool", bufs=4) as ypool,
            tc.tile_pool(name="warm", bufs=1) as warm,
            tc.tile_pool(name="pgu", bufs=1, space="PSUM") as pgu,
            tc.tile_pool(name="py", bufs=6, space="PSUM") as py,
        ):
            # ---- PE pre-warm: the HAM clock gate holds the PE at 1.2 GHz
            # until ~3.4us of sustained activity. The first real matmul waits
            # ~8us of DMA; burn that window on dummy matmuls so the real
            # stream starts at full 2.4 GHz.
            warm_w = warm.tile([P, P], bf16, tag="ww")
            warm_x = warm.tile([P, TT], bf16, tag="wx")
            nc.gpsimd.memset(warm_w, 0)
            nc.gpsimd.memset(warm_x, 0)
            for _ in range(14):
                p_warm = py.tile([P, HC], f32, tag="py")
                nc.tensor.matmul(p_warm, lhsT=warm_w, rhs=warm_x,
                                 start=True, stop=True)

            for tt in range(N_TT):
                tsl = slice(tt * TT, (tt + 1) * TT)
                # ---- fast start: a small first weight chunk (0.5MB) ahead
                # of the x chunks so the first matmul chain unblocks ASAP.
                if tt == 0:
                    wg0_t = warm.tile([P, N_H, P], bf16, tag="wg0")
                    nc.sync.dma_start(out=wg0_t, in_=wg_r[:, :, 0:P])
                x_ts = []
                x_t = xpool.tile([P, XCH, TT], bf16, tag="x")
                nc.sync.dma_start(out=x_t, in_=xT_r[:, 0:XCH, tsl])
                x_ts.append(x_t)
                wg_t = wpool.tile([P, N_H, WGD], bf16, tag="wg")
                nc.sync.dma_start(out=wg_t, in_=wg_r[:, :, 0:WGD])
                wu_t = wpool.tile([P, N_H, WGD], bf16, tag="wu")
                nc.sync.dma_start(out=wu_t, in_=wu_r[:, :, 0:WGD])
                for xc in range(1, N_XC):
                    x_t = xpool.tile([P, XCH, TT], bf16, tag="x")
                    nc.sync.dma_start(
                        out=x_t, in_=xT_r[:, xc * XCH:(xc + 1) * XCH, tsl])
                    x_ts.append(x_t)

                # ---- stage 1: gate/up + swiglu, d-tile at a time
                f_tiles = []
                for dt in range(N_D):
                    dw = dt % (WGD // P)   # position inside current weight load
                    if dw == 0 and dt > 0:
                        dsl = slice(dt * P, dt * P + WGD)
                        wg_t = wpool.tile([P, N_H, WGD], bf16, tag="wg")
                        wu_t = wpool.tile([P, N_H, WGD], bf16, tag="wu")
                        nc.sync.dma_start(out=wg_t, in_=wg_r[:, :, dsl])
                        nc.sync.dma_start(out=wu_t, in_=wu_r[:, :, dsl])
                    psum_g = pgu.tile([P, TT], f32, tag="pg")
                    psum_u = pgu.tile([P, TT], f32, tag="pu")
                    for h in range(N_H):
                        if tt == 0 and dt == 0:
                            wg_ap = wg0_t[:, h, :]
                        else:
                            wg_ap = wg_t[:, h, dw * P:(dw + 1) * P]
                        nc.tensor.matmul(
                            psum_g,
                            lhsT=wg_ap,
                            rhs=x_ts[h // XCH][:, h % XCH, :],
                            start=(h == 0), stop=(h == N_H - 1),
                        )
                    for h in range(N_H):
                        nc.tensor.matmul(
                            psum_u,
                            lhsT=wu_t[:, h, dw * P:(dw + 1) * P],
                            rhs=x_ts[h // XCH][:, h % XCH, :],
                            start=(h == 0), stop=(h == N_H - 1),
                        )
                    s_t = spool.tile([P, TT], f32, tag="s")
                    nc.scalar.activation(
                        out=s_t, in_=psum_g,
                        func=getattr(mybir.ActivationFunctionType, act),
                    )
                    f_t = fpool.tile([P, TT], bf16, tag="f")
                    nc.vector.tensor_mul(f_t, s_t, psum_u)
                    f_tiles.append(f_t)

                # ---- stage 2: y[t, h] = sum_d f_T[d, t] * wd[d, h]
                # ts loop inside the d-group loop: one wd tile alive at a
                # time, TS psum banks accumulate partials across d-groups.
                n_dg = N_D // WD_DCH
                for hc in range(N_HC):
                    psum_ys = [py.tile([P, HC], f32, name=f"py{i}", tag="py")
                               for i in range(TS)]
                    for dg in range(n_dg):
                        wd_t = wdpool.tile([P, WD_DCH, HC], bf16, tag="wd")
                        nc.sync.dma_start(
                            out=wd_t,
                            in_=wd_r[:, dg * WD_DCH:(dg + 1) * WD_DCH,
                                     hc * HC:(hc + 1) * HC],
                        )
                        for ts in range(TS):
                            for dc in range(WD_DCH):
                                dt = dg * WD_DCH + dc
                                nc.tensor.matmul(
                                    psum_ys[ts],
                                    lhsT=f_tiles[dt][:, ts * P:(ts + 1) * P],
                                    rhs=wd_t[:, dc, :],
                                    start=(dt == 0), stop=(dt == N_D - 1),
                                )
                    for ts in range(TS):
                        y_sb = ypool.tile([P, HC], f32, tag="y")
                        nc.vector.tensor_copy(y_sb, psum_ys[ts])
                        nc.sync.dma_start(
                            out=y_r[:, tt * TS + ts, hc * HC:(hc + 1) * HC],
                            in_=y_sb,
                        )
    if split_waits:
        _split_matmul_waits(nc)
    return nc


def _split_matmul_waits(nc):
    """walrus splits fp32r Matmult into LDW+MM and moves the Matmult's sync
    waits onto the generated LW struct, which has room for only one wait.
    Hoist every Matmult's waits onto a PE InstNoOp inserted just before it."""
    import concourse.mybir as mybir

    for f in nc.m.functions:
        for bb in f.blocks:
            insts = list(bb.instructions)
            out = []
            n_nops = 0
            for ins in insts:
                si = ins.sync_info
                tname = type(ins).__name__
                if (
                    si is not None
                    and len(si.on_wait) > (1 if tname != "InstMatmult" else 0)
                ):
                    keep = [] if tname == "InstMatmult" else [si.on_wait[-1]]
                    hoist = si.on_wait if tname == "InstMatmult" else si.on_wait[:-1]
                    for i, w in enumerate(hoist):
                        nop = mybir.InstNoOp(
                            name=f"{ins.name}-waitnop{i}",
                            engine=ins.engine,
                            ins=[],
                            outs=[],
                            sync_info=mybir.SyncInfo(
                                on_wait=[w], on_update=[]
                            ),
                        )
                        out.append(nop)
                        n_nops += 1
                    ins.sync_info = mybir.SyncInfo(
                        on_wait=keep, on_update=list(si.on_update)
                    )
                out.append(ins)
            if n_nops:
                bb.instructions = out


def make_in_maps(hidden_states, gate_proj, up_proj, down_proj):
    hs = np.ascontiguousarray(hidden_states, dtype=np.float32).reshape(E, T, H)
    in_maps = []
    for e in range(E):
        in_maps.append({
            "xT": np.ascontiguousarray(hs[e].T).astype(bf16),
            "wg": np.ascontiguousarray(gate_proj[e], dtype=np.float32).astype(bf16),
            "wu": np.ascontiguousarray(up_proj[e], dtype=np.float32).astype(bf16),
            "wd": np.ascontiguousarray(down_proj[e], dtype=np.float32).astype(bf16),
        })
    return in_maps


def kernel(hidden_states, gate_proj, up_proj, down_proj):
    from concourse.bass_utils import run_bass_kernel_spmd

    in_maps = make_in_maps(hidden_states, gate_proj, up_proj, down_proj)
    if "nc" not in _CACHE:
        _CACHE["nc"] = _build_bass()
    nc = _CACHE["nc"]

    res = run_bass_kernel_spmd(nc, in_maps, core_ids=list(range(E)))
    out = np.concatenate([res.results[e]["y"] for e in range(E)], axis=0)
    return out.astype(np.float32)


if __name__ == "__main__":
    # smoke: build only
    nc = _build_bass()
    print("built ok, instructions:", len(nc.inst_map))


# revision 14
# speedup vs baseline: 1.0048x; 1.0018x over previous
"""Trainium2 Bass kernel for Llama4TextExperts (MoE expert MLP chain).

Problem: E=8 experts, T=2048 tokens/expert, H=2048 hidden, D=4096 intermediate.
  hs (E*T, H) -> per expert e: g = hs_e @ Wg_e; u = hs_e @ Wu_e;
  f = u * silu(g); y_e = f @ Wd_e  -> out (E*T, H), all fp32.

Sharding: expert-parallel, 1 expert per NeuronCore (8 cores).

Per-core kernel design (all operands bf16, PSUM accumulate fp32):
  - Host pre-transposes hs_e -> xT [H, T] so the stage-1 moving operand has
    the contraction dim (H) on partitions.
  - Loop over T in tiles of TT=512 tokens:
      stage 1: for each of 32 d-tiles (128 wide): psum_g/psum_u accumulate
        16 matmuls over h-chunks (lhsT = W[h,d] 128x128 stationary bf16,
        rhs = xT[h, t] 128x512 moving bf16). silu on ScalarE,
        f = silu(g)*u on VectorE -> f_T[d] SBUF tiles [128(d) x 512(t)] bf16.
      stage 2: for each of 4 h-chunks (512 wide): for each of 4 t-subtiles
        (128): psum_y accumulates 32 matmuls over d (lhsT = f_T[d][:, ts]
        128x128 bf16 -> FWL fast weight load, rhs = wd[d, h] 128x512 bf16
        moving) -> copy -> DMA out.
  - Weights stream from HBM once per t-tile (50MB/t-tile, all bf16); DMA
    overlaps PE via double-buffered pools.
  - xT loads split into 4 chunks per t-tile so the first matmul can start
    after ~1.5MB of DMA instead of ~4MB.
"""

import os
import sys

for _p in ("/opt/trn_rl_repo",):
    if _p not in sys.path and os.path.isdir(_p):
        sys.path.insert(0, _p)

import numpy as np
from ml_dtypes import bfloat16 as bf16

E = 8
T = 2048
H = 2048
D = 4096

_CACHE = {}


def _build_bass(H_=H, D_=D, T_=T, TT=512, split_waits=True, act="Silu"):
    """Build the single-core Bass module (same program for all 8 cores)."""
    import concourse.bass as bass
    import concourse.mybir as mybir
    from concourse.tile import TileContext

    f32 = mybir.dt.float32
    bf16 = mybir.dt.bfloat16
    P = 128
    N_H = H_ // P            # h-chunks (16)
    N_D = D_ // P            # d-tiles (32)
    N_TT = T_ // TT          # t-tiles (4)
    TS = TT // P             # t-subtiles per t-tile (4)
    HC = 512                 # stage-2 output h-chunk width
    N_HC = H_ // HC          # 4
    WGD = 256                # wg/wu d-width per load (2 d-tiles)
    WD_DCH = 8               # wd d-chunks per load tile
    XCH = 4                  # h-chunks of x per load chunk
    N_XC = N_H // XCH        # 4 x-load chunks per t-tile

    nc = bass.Bass(trn_type="TRN2")

    xT = nc.declare_dram_parameter("xT", [H_, T_], bf16, isOutput=False)
    wg = nc.declare_dram_parameter("wg", [H_, D_], bf16, isOutput=False)
    wu = nc.declare_dram_parameter("wu", [H_, D_], bf16, isOutput=False)
    wd = nc.declare_dram_parameter("wd", [D_, H_], bf16, isOutput=False)
    y = nc.declare_dram_parameter("y", [T_, H_], f32, isOutput=True)

    xT_r = xT[:].rearrange("(n p) t -> p n t", p=P)    # [128, N_H, T]
    wg_r = wg[:].rearrange("(n p) d -> p n d", p=P)    # [128, N_H, D]
    wu_r = wu[:].rearrange("(n p) d -> p n d", p=P)
    wd_r = wd[:].rearrange("(n p) h -> p n h", p=P)    # [128, N_D, H]
    y_r = y[:].rearrange("(n p) h -> p n h", p=P)      # [128, T//128, H]

    with TileContext(nc) as tc:
        with (
            tc.tile_pool(name="xpool", bufs=N_XC + 1) as xpool,
            tc.tile_pool(name="wpool", bufs=2) as wpool,
            tc.tile_pool(name="wdpool", bufs=4) as wdpool,
            tc.tile_pool(name="fpool", bufs=N_D + 2) as fpool,
            tc.tile_pool(name="spool", bufs=3) as spool,
            tc.tile_pool(name="ypool", bufs=4) as ypool,
            tc.tile_pool(name="warm", bufs=1) as warm,
            tc.tile_pool(name="pgu", bufs=1, space="PSUM") as pgu,
            tc.tile_pool(name="py", bufs=6, space="PSUM") as py,
        ):
            # ---- PE pre-warm: the HAM clock gate holds the PE at 1.2 GHz
            # until ~3.4us of sustained activity. The first real matmul waits
            # ~8us of DMA; burn that window on dummy matmuls so the real
            # stream starts at full 2.4 GHz.
            warm_w = warm.tile([P, P], bf16, tag="ww")
            warm_x = warm.tile([P, TT], bf16, tag="wx")
            nc.gpsimd.memset(warm_w, 0)
            nc.gpsimd.memset(warm_x, 0)
            for _ in range(14):
                p_warm = py.tile([P, HC], f32, tag="py")
                nc.tensor.matmul(p_warm, lhsT=warm_w, rhs=warm_x,
                                 start=True, stop=True)

            for tt in range(N_TT):
                tsl = slice(tt * TT, (tt + 1) * TT)
                # ---- fast start: first weight chunk interleaved with the
                # x chunks so the first matmul chain unblocks ASAP.
                wg_t = wpool.tile([P, N_H, WGD], bf16, tag="wg")
                nc.sync.dma_start(out=wg_t, in_=wg_r[:, :, 0:WGD])
                x_ts = []
                x_t = xpool.tile([P, XCH, TT], bf16, tag="x")
                nc.sync.dma_start(out=x_t, in_=xT_r[:, 0:XCH, tsl])
                x_ts.append(x_t)
                wu_t = wpool.tile([P, N_H, WGD], bf16, tag="wu")
                nc.sync.dma_start(out=wu_t, in_=wu_r[:, :, 0:WGD])
                for xc in range(1, N_XC):
                    x_t = xpool.tile([P, XCH, TT], bf16, tag="x")
                    nc.sync.dma_start(
                        out=x_t, in_=xT_r[:, xc * XCH:(xc + 1) * XCH, tsl])
                    x_ts.append(x_t)

                # ---- stage 1: gate/up + swiglu, d-tile at a time
                f_tiles = []
                for dt in range(N_D):
                    dw = dt % (WGD // P)   # position inside current weight load
                    if dw == 0 and dt > 0:
                        dsl = slice(dt * P, dt * P + WGD)
                        wg_t = wpool.tile([P, N_H, WGD], bf16, tag="wg")
                        wu_t = wpool.tile([P, N_H, WGD], bf16, tag="wu")
                        nc.sync.dma_start(out=wg_t, in_=wg_r[:, :, dsl])
                        nc.sync.dma_start(out=wu_t, in_=wu_r[:, :, dsl])
                    psum_g = pgu.tile([P, TT], f32, tag="pg")
                    psum_u = pgu.tile([P, TT], f32, tag="pu")
                    for h in range(N_H):
                        nc.tensor.matmul(
                            psum_g,
                            lhsT=wg_t[:, h, dw * P:(dw + 1) * P],
                            rhs=x_ts[h // XCH][:, h % XCH, :],
                            start=(h == 0), stop=(h == N_H - 1),
                        )
                    for h in range(N_H):
                        nc.tensor.matmul(
                            psum_u,
                            lhsT=wu_t[:, h, dw * P:(dw + 1) * P],
                            rhs=x_ts[h // XCH][:, h % XCH, :],
                            start=(h == 0), stop=(h == N_H - 1),
                        )
                    s_t = spool.tile([P, TT], f32, tag="s")
                    nc.scalar.activation(
                        out=s_t, in_=psum_g,
                        func=getattr(mybir.ActivationFunctionType, act),
                    )
                    f_t = fpool.tile([P, TT], bf16, tag="f")
                    nc.vector.tensor_mul(f_t, s_t, psum_u)
                    f_tiles.append(f_t)

                # ---- stage 2: y[t, h] = sum_d f_T[d, t] * wd[d, h]
                # ts loop inside the d-group loop: one wd tile alive at a
                # time, TS psum banks accumulate partials across d-groups.
                n_dg = N_D // WD_DCH
                for hc in range(N_HC):
                    psum_ys = [py.tile([P, HC], f32, name=f"py{i}", tag="py")
                               for i in range(TS)]
                    for dg in range(n_dg):
                        wd_t = wdpool.tile([P, WD_DCH, HC], bf16, tag="wd")
                        nc.sync.dma_start(
                            out=wd_t,
                            in_=wd_r[:, dg * WD_DCH:(dg + 1) * WD_DCH,
                                     hc * HC:(hc + 1) * HC],
                        )
                        for ts in range(TS):
                            for dc in range(WD_DCH):
                                dt = dg * WD_DCH + dc
                                nc.tensor.matmul(
                                    psum_ys[ts],
                                    lhsT=f_tiles[dt][:, ts * P:(ts + 1) * P],
                                    rhs=wd_t[:, dc, :],
                                    start=(dt == 0), stop=(dt == N_D - 1),
                                )
                    for ts in range(TS):
                        y_sb = ypool.tile([P, HC], f32, tag="y")
                        nc.vector.tensor_copy(y_sb, psum_ys[ts])
                        nc.sync.dma_start(
                            out=y_r[:, tt * TS + ts, hc * HC:(hc + 1) * HC],
                            in_=y_sb,
                        )
    if split_waits:
        _split_matmul_waits(nc)
    return nc


def _split_matmul_waits(nc):
    """walrus splits fp32r Matmult into LDW+MM and moves the Matmult's sync
    waits onto the generated LW struct, which has room for only one wait.
    Hoist every Matmult's waits onto a PE InstNoOp inserted just before it."""
    import concourse.mybir as mybir

    for f in nc.m.functions:
        for bb in f.blocks:
            insts = list(bb.instructions)
            out = []
            n_nops = 0
            for ins in insts:
                si = ins.sync_info
                tname = type(ins).__name__
                if (
                    si is not None
                    and len(si.on_wait) > (1 if tname != "InstMatmult" else 0)
                ):
                    keep = [] if tname == "InstMatmult" else [si.on_wait[-1]]
                    hoist = si.on_wait if tname == "InstMatmult" else si.on_wait[:-1]
                    for i, w in enumerate(hoist):
                        nop = mybir.InstNoOp(
                            name=f"{ins.name}-waitnop{i}",
                            engine=ins.engine,
                            ins=[],
                            outs=[],
                            sync_info=mybir.SyncInfo(
                                on_wait=[w], on_update=[]
                            ),
                        )
                        out.append(nop)
                        n_nops += 1
                    ins.sync_info = mybir.SyncInfo(
                        on_wait=keep, on_update=list(si.on_update)
                    )
                out.append(ins)
            if n_nops:
                bb.instructions = out


def make_in_maps(hidden_states, gate_proj, up_proj, down_proj):
    hs = np.ascontiguousarray(hidden_states, dtype=np.float32).reshape(E, T, H)
    in_maps = []
    for e in range(E):
        in_maps.append({
            "xT": np.ascontiguousarray(hs[e].T).astype(bf16),
            "wg": np.ascontiguousarray(gate_proj[e], dtype=np.float32).astype(bf16),
            "wu": np.ascontiguousarray(up_proj[e], dtype=np.float32).astype(bf16),
            "wd": np.ascontiguousarray(down_proj[e], dtype=np.float32).astype(bf16),
        })
    return in_maps


def kernel(hidden_states, gate_proj, up_proj, down_proj):
    from concourse.bass_utils import run_bass_kernel_spmd

    in_maps = make_in_maps(hidden_states, gate_proj, up_proj, down_proj)
    if "nc" not in _CACHE:
        _CACHE["nc"] = _build_bass()
    nc = _CACHE["nc"]

    res = run_bass_kernel_spmd(nc, in_maps, core_ids=list(range(E)))
    out = np.concatenate([res.results[e]["y"] for e in range(E)], axis=0)
    return out.astype(np.float32)


if __name__ == "__main__":
    # smoke: build only
    nc = _build_bass()
    print("built ok, instructions:", len(nc.inst_map))
